# revision 2
# baseline (speedup 1.0000x reference)
"""BitLinear inference kernel for Trainium2, sharded over 8 NeuronCores.

Computes, per the reference:
    w_q = sign(w - mean(w));  w_scale = mean(|w|)
    b_q = sign(b - mean(b));  b_scale = mean(|b|)
    xn  = x / max(||x||_2, 1e-12) * D**-0.5            (per token)
    sc  = 127 / max(max|xn|, 1e-5)                     (per token)
    x_q = clip(round(xn * sc), -128, 127)
    y   = (x_q @ w_q.T + b_q) / (w_scale * sc * b_scale)

Sharding: x/y split into 8 contiguous row blocks of 4096 tokens (data
parallel over B*S); w, b replicated.  All per-token math is on-core.

Implementation notes (measured on HW via repeat-loop differencing):
  - round(xn*sc) == round(x * 127/amax|x|) mathematically (the l2 norm
    cancels); x_q exact via the +-1.5*2^23 magic-constant round, carried
    in bf16 (integers |v|<=127 exact), w_q {-1,0,1} in bf16 -> the PE
    matmul accumulation in f32 PSUM is exact vs the reference einsum.
  - The kernel is NOT PE-bound: the elementwise passes over [128,1024]
    tiles (sumsq, amax, quant x2, transpose copy-back, epilogue) are.
    They are split across ACT (sumsq-square, t1 = m*x + MAGIC fused in
    one activation, epilogue scale) and DVE (amax, magic-subtract,
    PSUM->SBUF transpose copy-back).
  - x_q transposed per tile on the PE (8x 128x128 bf16 transposes into
    PSUM + one DVE copy back).  CRITICAL: the transpose staging pool
    needs 3 PSUM banks (xpsbufs=3) — with 2, PE transposes serialize
    against the DVE copies and the whole kernel runs ~2x slower.
  - bias b_q added via a K=1 rank-1 matmul opening every PSUM group
    (cheaper on the idle-ish PE than a DVE tensor add from PSUM).
  - y is stored as bf16 and upcast on the host: the dequantized values
    carry >= 8 exact integer bits through the scale, so bf16 adds
    <= 0.4% relative error against a 2e-2 gate, and halves store DMA.
  - dequant scale needs 1/||x||: DVE reciprocal + ACT sqrt seed + two
    Newton rsqrt refinements (ACT sqrt alone is too inaccurate).
"""

import os
import sys

import numpy as np

for _p in ("/opt/trn_rl_repo", "/root/.axon_site/_ro/trn_rl_repo"):
    if os.path.isdir(_p) and _p not in sys.path:
        sys.path.insert(0, _p)

import concourse.bacc as bacc
import concourse.tile as tile
from concourse import mybir
from concourse.bass_utils import run_bass_kernel_spmd
from concourse.masks import make_identity

F32 = mybir.dt.float32
BF16 = mybir.dt.bfloat16
ALU = mybir.AluOpType
ACTF = mybir.ActivationFunctionType

N_CORES = 8
B, S, D, O = 4, 8192, 1024, 1024
TOKENS = B * S
TOK_PER_CORE = TOKENS // N_CORES          # 4096
P = 128
NTILES = TOK_PER_CORE // P                # 32
DCH = D // P                              # 8
OCH = O // P                              # 8

MAGIC = 1.5 * 2.0**23                     # round-to-nearest-even constant
DIM_SCALE = float(D) ** -0.5
EPS_NORM_SQ = 1e-24                       # (1e-12)**2, matches l2 clamp
EPS_SCALE = 1e-5

DEFAULT_CFG = dict(
    group=4,           # token tiles per x-load DMA
    xpsbufs=3,         # PSUM staging banks for the x_q transposes
    deep=False,        # deeper SBUF pools
)


def build_module(repeat: int = 1, cfg: dict | None = None):
    c = dict(DEFAULT_CFG)
    if cfg:
        c.update(cfg)
    return _build(repeat, c)


def _build(repeat: int, c: dict):
    GROUP = c["group"]
    NGROUPS = NTILES // GROUP

    nc = bacc.Bacc("TRN2", target_bir_lowering=False, debug=False)

    x_d = nc.dram_tensor("x", [TOK_PER_CORE, D], F32, kind="ExternalInput")
    w_d = nc.dram_tensor("w", [O, D], F32, kind="ExternalInput")
    b_d = nc.dram_tensor("b", [O], F32, kind="ExternalInput")
    y_d = nc.dram_tensor("y", [TOK_PER_CORE, O], BF16, kind="ExternalOutput")

    x_r = x_d.ap().rearrange("(a p) d -> p a d", p=P)   # [128, 32, 1024]
    y_r = y_d.ap().rearrange("(a p) d -> p a d", p=P)
    w_r = w_d.ap().rearrange("(r p) d -> p r d", p=P)   # [128, 8, 1024]
    b_r = b_d.ap().rearrange("(o d) -> o d", o=1)       # [1, 1024]

    with tile.TileContext(nc) as tc:
        import contextlib

        with contextlib.ExitStack() as ctx:
            dp = 1 if c["deep"] else 0
            consts = ctx.enter_context(tc.tile_pool(name="consts", bufs=1))
            wpool = ctx.enter_context(tc.tile_pool(name="wpool", bufs=1))
            wtpool = ctx.enter_context(tc.tile_pool(name="wtpool", bufs=1))
            xpool = ctx.enter_context(
                tc.tile_pool(name="xpool", bufs=(3 if GROUP <= 4 else 2) + dp)
            )
            scr = ctx.enter_context(tc.tile_pool(name="scr", bufs=2 + dp))
            tpool = ctx.enter_context(tc.tile_pool(name="tpool", bufs=3 + dp))
            qpool = ctx.enter_context(
                tc.tile_pool(name="qpool", bufs=4 + 2 * dp)
            )
            xtpool = ctx.enter_context(
                tc.tile_pool(name="xtpool", bufs=6 + 2 * dp)
            )
            ypool = ctx.enter_context(tc.tile_pool(name="ypool", bufs=3 + dp))
            stats = ctx.enter_context(tc.tile_pool(name="stats", bufs=3 + dp))
            pspool = ctx.enter_context(
                tc.tile_pool(name="pspool", bufs=2, space="PSUM")
            )
            # prep-only w-transpose staging: [128, 512] f32 = 1 bank
            wps = ctx.enter_context(
                tc.tile_pool(name="wps", bufs=1, space="PSUM")
            )
            xps = ctx.enter_context(
                tc.tile_pool(name="xps", bufs=c["xpsbufs"], space="PSUM")
            )

            # ---------------- constants ----------------
            identity = consts.tile([P, P], F32)
            make_identity(nc, identity)
            identity_bf = consts.tile([P, P], BF16)
            make_identity(nc, identity_bf)
            ones_row = consts.tile([1, P], BF16)
            nc.vector.memset(ones_row, 1.0)
            ones128 = consts.tile([P, P], F32)
            nc.vector.memset(ones128, 1.0)
            ones_col_f = consts.tile([1, P], F32)
            nc.vector.memset(ones_col_f, 1.0)

            # ---------------- weight/bias prep ----------------
            def emit_prep():
                # bias first so b_q is ready before the first PSUM group
                b_sb = consts.tile([1, O], F32)
                nc.sync.dma_start(out=b_sb, in_=b_r)

                w_sb = wpool.tile([P, OCH, D], F32)
                for half in range(4):
                    nc.sync.dma_start(
                        out=w_sb[:, half * 2 : half * 2 + 2, :],
                        in_=w_r[:, half * 2 : half * 2 + 2, :],
                    )

                # sum(w) on ACT+DVE split; sum|w| on DVE
                wsum = consts.tile([P, OCH], F32)
                wabs = consts.tile([P, OCH], F32)
                for r in range(OCH):
                    if r % 2 == 0:
                        dump = scr.tile([P, D], F32, tag="wdump")
                        nc.scalar.activation(
                            out=dump, in_=w_sb[:, r, :], func=ACTF.Copy,
                            accum_out=wsum[:, r : r + 1],
                        )
                    else:
                        nc.vector.tensor_reduce(
                            out=wsum[:, r : r + 1], in_=w_sb[:, r, :],
                            axis=mybir.AxisListType.X, op=ALU.add,
                        )
                for r in range(OCH):
                    nc.vector.tensor_reduce(
                        out=wabs[:, r : r + 1], in_=w_sb[:, r, :],
                        axis=mybir.AxisListType.X, op=ALU.add,
                        apply_absolute_value=True,
                    )
                w12 = consts.tile([P, 2], F32)
                nc.vector.tensor_reduce(
                    out=w12[:, 0:1], in_=wsum, axis=mybir.AxisListType.X,
                    op=ALU.add,
                )
                nc.vector.tensor_reduce(
                    out=w12[:, 1:2], in_=wabs, axis=mybir.AxisListType.X,
                    op=ALU.add,
                )
                # cross-partition reduce + broadcast via f32 ones-matmul
                statps = xps.tile([P, 4], F32, tag="xtp", name="statps")
                nc.tensor.matmul(
                    statps[:, 0:2], lhsT=ones128, rhs=w12,
                    start=True, stop=True,
                )
                neg_mean_w = consts.tile([P, 1], F32)
                w_scale = consts.tile([P, 1], F32)
                nc.vector.tensor_scalar(
                    out=neg_mean_w, in0=statps[:, 0:1],
                    scalar1=-1.0 / float(O * D), scalar2=None, op0=ALU.mult,
                )
                nc.vector.tensor_scalar(
                    out=w_scale, in0=statps[:, 1:2],
                    scalar1=1.0 / float(O * D), scalar2=None, op0=ALU.mult,
                )

                # transpose raw w on PE, then Sign(wT - mean) on ACT
                wqT = wtpool.tile([P, DCH, O], BF16)
                for ch in range(DCH):
                    for half in range(2):
                        pt = wps.tile([P, 512], F32, tag="wtp")
                        for r4 in range(4):
                            r = half * 4 + r4
                            nc.tensor.transpose(
                                pt[:, r4 * P : (r4 + 1) * P],
                                w_sb[:, r, ch * P : (ch + 1) * P],
                                identity,
                            )
                        nc.scalar.activation(
                            out=wqT[:, ch, half * 512 : (half + 1) * 512],
                            in_=pt, func=ACTF.Sign,
                            bias=neg_mean_w, scale=1.0,
                        )

                # bias quant
                bsum = consts.tile([1, 1], F32)
                babs = consts.tile([1, 1], F32)
                nc.vector.tensor_reduce(
                    out=bsum, in_=b_sb, axis=mybir.AxisListType.X, op=ALU.add
                )
                nc.vector.tensor_reduce(
                    out=babs, in_=b_sb, axis=mybir.AxisListType.X, op=ALU.add,
                    apply_absolute_value=True,
                )
                neg_mean_b = consts.tile([1, 1], F32)
                b_scale1 = consts.tile([1, 1], F32)
                nc.vector.tensor_scalar(
                    out=neg_mean_b, in0=bsum, scalar1=-1.0 / float(O),
                    scalar2=None, op0=ALU.mult,
                )
                nc.vector.tensor_scalar(
                    out=b_scale1, in0=babs, scalar1=1.0 / float(O),
                    scalar2=None, op0=ALU.mult,
                )
                bq = consts.tile([1, O], BF16)
                nc.scalar.activation(
                    out=bq, in_=b_sb, func=ACTF.Sign, bias=neg_mean_b,
                    scale=1.0,
                )

                # invc = 1 / (127 * w_scale * b_scale), broadcast to [128,1]
                bps = xps.tile([P, 1], F32, tag="xtp", name="bps")
                nc.tensor.matmul(
                    bps, lhsT=ones_col_f, rhs=b_scale1, start=True, stop=True
                )
                wb = consts.tile([P, 1], F32)
                nc.vector.tensor_tensor(
                    out=wb, in0=w_scale, in1=bps, op=ALU.mult
                )
                wb127 = consts.tile([P, 1], F32)
                nc.vector.tensor_scalar(
                    out=wb127, in0=wb, scalar1=127.0, scalar2=None,
                    op0=ALU.mult,
                )
                invc = consts.tile([P, 1], F32)
                nc.vector.reciprocal(out=invc, in_=wb127)
                return wqT, bq, invc

            # ---------------- main loop ----------------
            def emit_group(g, prep):
                wqT, bq, invc = prep
                xg = xpool.tile([P, GROUP, D], F32)
                nc.sync.dma_start(
                    out=xg, in_=x_r[:, g * GROUP : (g + 1) * GROUP, :]
                )

                sumsq = stats.tile([P, GROUP], F32)
                amax = stats.tile([P, GROUP], F32)
                for j in range(GROUP):
                    # sum(x^2) on ACT (Square with add-accumulate)
                    sq = scr.tile([P, D], F32, tag="sq")
                    nc.scalar.activation(
                        out=sq, in_=xg[:, j, :], func=ACTF.Square,
                        accum_out=sumsq[:, j : j + 1],
                    )
                    nc.vector.tensor_reduce(
                        out=amax[:, j : j + 1], in_=xg[:, j, :],
                        axis=mybir.AxisListType.X, op=ALU.max,
                        apply_absolute_value=True,
                    )

                # per-token scalar chain on [128, GROUP]
                m = stats.tile([P, GROUP], F32)
                gsc = stats.tile([P, GROUP], F32)
                ssq = stats.tile([P, GROUP], F32)
                nc.vector.tensor_scalar(
                    out=ssq, in0=sumsq, scalar1=EPS_NORM_SQ, scalar2=None,
                    op0=ALU.max,
                )
                u = stats.tile([P, GROUP], F32)
                nc.vector.reciprocal(out=u, in_=ssq)
                v = stats.tile([P, GROUP], F32)
                nc.scalar.activation(out=v, in_=u, func=ACTF.Sqrt)
                for _ in range(2):  # Newton rsqrt refinement
                    rr = stats.tile([P, GROUP], F32, tag="rr")
                    nc.vector.tensor_tensor(out=rr, in0=v, in1=v, op=ALU.mult)
                    qq = stats.tile([P, GROUP], F32, tag="qq")
                    nc.vector.tensor_tensor(
                        out=qq, in0=rr, in1=ssq, op=ALU.mult
                    )
                    ww = stats.tile([P, GROUP], F32, tag="ww")
                    nc.vector.tensor_scalar(
                        out=ww, in0=qq, scalar1=-0.5, scalar2=1.5,
                        op0=ALU.mult, op1=ALU.add,
                    )
                    v2 = stats.tile([P, GROUP], F32, tag="vv")
                    nc.vector.tensor_tensor(out=v2, in0=v, in1=ww, op=ALU.mult)
                    v = v2

                am = stats.tile([P, GROUP], F32)
                nc.vector.tensor_scalar(
                    out=am, in0=amax, scalar1=1e-30, scalar2=None, op0=ALU.max
                )
                im = stats.tile([P, GROUP], F32)
                nc.vector.reciprocal(out=im, in_=am)
                nc.vector.tensor_scalar(
                    out=m, in0=im, scalar1=127.0, scalar2=None, op0=ALU.mult
                )
                ax1 = stats.tile([P, GROUP], F32)
                nc.vector.tensor_tensor(out=ax1, in0=amax, in1=v, op=ALU.mult)
                axnc = stats.tile([P, GROUP], F32)
                nc.vector.tensor_scalar(
                    out=axnc, in0=ax1, scalar1=DIM_SCALE, scalar2=EPS_SCALE,
                    op0=ALU.mult, op1=ALU.max,
                )
                nc.vector.tensor_scalar(
                    out=gsc, in0=axnc, scalar1=invc, scalar2=None, op0=ALU.mult
                )

                for j in range(GROUP):
                    t = g * GROUP + j
                    # quantize: t1 = m*x + MAGIC fused on ACT; subtract on DVE
                    xq = qpool.tile([P, D], BF16)
                    t1 = tpool.tile([P, D], F32)
                    nc.scalar.activation(
                        out=t1, in_=xg[:, j, :], func=ACTF.Copy,
                        bias=MAGIC, scale=m[:, j : j + 1],
                    )
                    nc.vector.tensor_scalar(
                        out=xq, in0=t1, scalar1=MAGIC, scalar2=None,
                        op0=ALU.subtract,
                    )

                    # transpose xq -> xqT on PE (PSUM stage + DVE copy back)
                    xqT = xtpool.tile([P, DCH, P], BF16)
                    ptx = xps.tile([P, D], BF16, tag="xtp")
                    for ch in range(DCH):
                        nc.tensor.transpose(
                            ptx[:, ch * P : (ch + 1) * P],
                            xq[:, ch * P : (ch + 1) * P],
                            identity_bf,
                        )
                    xqT_flat = xqT.rearrange("p c t -> p (c t)")
                    nc.vector.tensor_copy(out=xqT_flat, in_=ptx)

                    # matmul: y = x_q @ w_q.T + b_q  (PSUM f32, exact)
                    ps = pspool.tile([P, O], F32, tag="ps")
                    pss = [ps[:, 0:512], ps[:, 512:1024]]
                    for h in range(2):
                        nc.tensor.matmul(
                            pss[h], lhsT=ones_row,
                            rhs=bq[:, h * 512 : (h + 1) * 512],
                            start=True, stop=False,
                        )
                    # d-chunk outer, o-half inner: adjacent MMs share lhsT
                    for ch in range(DCH):
                        for h in range(2):
                            nc.tensor.matmul(
                                pss[h],
                                lhsT=xqT[:, ch, :],
                                rhs=wqT[:, ch, h * 512 : (h + 1) * 512],
                                start=False,
                                stop=(ch == DCH - 1),
                            )

                    # epilogue: ps * gsc -> y (bf16), batched stores x2
                    if j % 2 == 0:
                        yt2 = ypool.tile([P, 2, O], BF16, tag="yt")
                    nc.scalar.activation(
                        out=yt2[:, j % 2, :], in_=ps, func=ACTF.Copy,
                        bias=0.0, scale=gsc[:, j : j + 1],
                    )
                    if j % 2 == 1:
                        nc.sync.dma_start(
                            out=y_r[:, t - 1 : t + 1, :], in_=yt2
                        )

            def main_loop(prep):
                for g in range(NGROUPS):
                    emit_group(g, prep)

            if repeat == 1:
                prep = emit_prep()
                main_loop(prep)
            else:
                prep = emit_prep()
                with tc.For_i(0, repeat, 1):
                    main_loop(prep)

    nc.compile()
    return nc


_NC_CACHE = None


def _get_module():
    global _NC_CACHE
    if _NC_CACHE is None:
        _NC_CACHE = build_module()
    return _NC_CACHE


def kernel(x: np.ndarray, w: np.ndarray, b: np.ndarray) -> np.ndarray:
    assert x.shape == (B, S, D) and w.shape == (O, D) and b.shape == (O,)
    nc = _get_module()

    xf = np.ascontiguousarray(x.reshape(TOKENS, D), dtype=np.float32)
    w = np.ascontiguousarray(w, dtype=np.float32)
    b = np.ascontiguousarray(b, dtype=np.float32)

    in_maps = [
        {
            "x": xf[i * TOK_PER_CORE : (i + 1) * TOK_PER_CORE],
            "w": w,
            "b": b,
        }
        for i in range(N_CORES)
    ]
    res = run_bass_kernel_spmd(nc, in_maps, core_ids=list(range(N_CORES)))
    out = np.concatenate(
        [np.asarray(res.results[i]["y"]) for i in range(N_CORES)], axis=0
    )
    return out.reshape(B, S, O).astype(np.float32)
